# revision 22
# baseline (speedup 1.0000x reference)
"""BitConv2d (ternary-weight 3x3 conv, power-of-two rescale) on 8 TRN2 NeuronCores.

Strategy:
  - Data-parallel over batch: 32 images -> 4 per core (2 image pairs).
  - fp16 input transfer (quantization makes the fp16 cast nearly lossless:
    measured end-to-end rel err ~3e-3 vs 2e-2 tolerance).  Loads arrive in
    28-row parts (201KB DMAs, 6272B lines) into per-part staging subtiles so
    quantization starts after the first part, not the whole image.
  - Activation quantization x_int = clip(round(clip(x,-1,1)/2^-6), -127, 127):
      i16 = RNE(64*x + 128) on GPSIMD (hw cast rounds to nearest even),
      bf16 = clip(i16, 64, 192) -> v = x_int + 128 (exact ints), alternating
      DVE / GPSIMD per chunk to balance engine load.
    The +128 offset keeps values positive; padded border cells are memset to
    128 so the offset contributes exactly 128*sum(w) per output channel,
    which is folded into the bias on the host.  xq is split into 7 subtiles
    of 16 rows (+2 halo) per pair so conv iterations start as soon as their
    rows are quantized.
  - Conv as 9 accumulating matmuls per output tile (K=Cin=64, M=Cout=64),
    packed 4-per-array with tile_position quadrants:
      rows 0-63   = image A channels, rows 64-127 = image B channels
      cols 0-63   = output row-block r, cols 64-127 = row-block r+1.
    PSUM pairing is per image: ps_A[0:64] = (A, blk r), ps_A[64:128] =
    (A, blk r+1) so the epilogue + store run on full 128-partition tiles.
  - Epilogue y = psum * 2^(act_exp+s_exp[c]) + bias'[c] -> fp16; image A on
    DVE, image B on ACT.  Two iterations share one output tile so stores
    move 16 rows (229KB) each.
  - Loads on the sync-engine HWDGE ring, stores on the scalar-engine ring
    (dedicated directions; mixing them FIFO-blocks the ring).  Stores use a
    blocked dram layout so the AP has a 128-wide outer dim (all 16 SDMA
    engines; a narrow outer dim leaves most engines idle).
  - Teardown sem RANGE_CLEARs distributed across all 5 engines (a single
    engine walks ~25ns/sem -> ~7us serial for ~270 sems).
Output returned as float32 (host upcast + untangle of the fp16 device output).
"""

import numpy as np
import ml_dtypes
from contextlib import ExitStack

_NC_CACHE = {}

N_CORES = 8
H = W = 112
HP = H + 2  # padded width
CIN = COUT = 64
P = 128
IMGS_PER_CORE = 4
LOAD_ROWS = 28                # rows per input load part (4 parts per image)
QUANT_ROWS = 14               # rows per quantization chunk (2 per load part)
ROWS_PER_TILE = 4             # output rows per matmul tile (N = 4*112 = 448)
NFREE = ROWS_PER_TILE * W     # 448
XQ_ROWS = 16                  # img rows per xq subtile (2 conv iters)
XQ_PAD = XQ_ROWS + 2          # + halo


def _patch_bass_and_tile(bass_mod, tile_mod):
    """Build-time patches for this walrus build:
      1. split the final Tile drain's multi-wait into single-wait nops;
      2. distribute semaphore RANGE_CLEARs across all engines (they are
         emitted serially on gpsimd otherwise and cost ~25ns/sem)."""
    from concourse.vector_clock import ScopedClock, VectorClock
    from concourse.bass import compact_to_ranges, SemaphoreHandle

    if getattr(tile_mod.TileContext, "_drain_patched", False):
        return

    def clear_distributed(self, sems):
        if not sems:
            return
        sem_nums = [
            sem.num if isinstance(sem, SemaphoreHandle) else sem for sem in sems
        ]
        sem_ranges = compact_to_ranges(sem_nums)
        # only lightly-loaded engines: a clear emitted on tensor/scalar lands
        # in that engine's instruction stream and stalls matmul/epilogue issue
        # when tile scopes recycle semaphores mid-run.
        engs = [self.sync, self.gpsimd, self.vector]
        per = max(1, (len(sem_nums) + len(engs) - 1) // len(engs))
        ei = cnt = 0
        for sem_range in sem_ranges:
            assert self._state.free_isdisjoint(sem_range)
            x = sem_range.start
            while x < sem_range.stop:
                take = min(sem_range.stop - x, per - cnt)
                sub = range(x, x + take)
                engs[ei].drain(semaphore_range=sub)
                engs[ei].sem_clear(sub)
                x += take
                cnt += take
                if cnt >= per and ei < len(engs) - 1:
                    ei += 1
                    cnt = 0
        self._state.prepend_free_semaphores(sem_nums)
        for poison_set in self._tile_sem_poison_stack:
            poison_set.update(sem_nums)

    bass_mod.Bass.clear_and_free_semaphores = clear_distributed

    def _drain_and_barrier_split(self, tick_clock, wait_clock):
        vclock = tick_clock.global_clock
        n = len(vclock)
        for proc in range(n):
            t = vclock[proc]
            if t <= 0:
                continue
            vec = [0] * n
            vec[proc] = t
            nop = self.nc.sync.nop()
            wait_clock.add_sem_waits(nop.ins, ScopedClock({None: VectorClock(vec)}))
        self.nc.sync.drain()
        assert self.sems is not None
        popped = self.nc._tile_sem_poison_stack.pop()
        assert popped is self._sem_poison
        self.nc.all_engine_barrier()
        self.nc.clear_and_free_semaphores(list(self.sems.allocated().values()))
        self.nc.all_engine_barrier()

    tile_mod.TileContext._drain_and_barrier = _drain_and_barrier_split
    tile_mod.TileContext._drain_patched = True


def _split_multi_syncs(nc):
    """This walrus build accepts at most ONE sync wait (and one update) per
    instruction.  Hoist extra waits onto preceding nops and extra updates onto
    following nops (same engine, so ordering semantics are preserved)."""
    import concourse.mybir as mybir

    fn = nc.m.functions[0]
    ctr = 0
    for bb in fn.blocks:
        new_insts = []
        for inst in bb.instructions:
            si = inst.sync_info
            pre, post = [], []
            if si is not None and si.on_wait and len(si.on_wait) > 1:
                for w in list(si.on_wait[:-1]):
                    ctr += 1
                    pre.append(
                        mybir.InstNoOp(
                            name=f"wsplit_nop_{ctr}",
                            engine=inst.engine,
                            sync_info=mybir.SyncInfo(on_wait=[w], on_update=[]),
                        )
                    )
                si.on_wait = [si.on_wait[-1]]
            if (
                si is not None
                and si.on_update
                and len(si.on_update) > 1
                and not isinstance(inst, (mybir.InstDMACopy, mybir.InstDMA))
            ):
                for u in list(si.on_update[1:]):
                    ctr += 1
                    post.append(
                        mybir.InstNoOp(
                            name=f"usplit_nop_{ctr}",
                            engine=inst.engine,
                            sync_info=mybir.SyncInfo(on_wait=[], on_update=[u]),
                        )
                    )
                si.on_update = [si.on_update[0]]
            new_insts.extend(pre)
            new_insts.append(inst)
            new_insts.extend(post)
        if len(new_insts) != len(bb.instructions):
            bb.instructions[:] = new_insts
    for bb in fn.blocks:
        for inst in bb.instructions:
            if inst.name.startswith(("wsplit_nop_", "usplit_nop_")):
                if inst.name not in nc.inst_map:
                    nc.register_instruction(inst)
    return ctr


def build_nc(repeat: int = 1):
    import concourse.bass as bass
    import concourse.mybir as mybir
    import concourse.tile as tile

    _patch_bass_and_tile(bass, tile)

    f32 = mybir.dt.float32
    f16 = mybir.dt.float16
    bf16 = mybir.dt.bfloat16
    i16 = mybir.dt.int16
    Alu = mybir.AluOpType
    Act = mybir.ActivationFunctionType

    nc = bass.Bass(trn_type="TRN2")
    x4 = nc.dram_tensor("x4", (IMGS_PER_CORE, CIN, H, W), f16, kind="ExternalInput")
    wsb = nc.dram_tensor("wsb", (P, 9 * COUT), bf16, kind="ExternalInput")
    sb = nc.dram_tensor("sb", (P, 2), f32, kind="ExternalInput")

    n_parts = H // LOAD_ROWS                # 4 load parts per image
    n_chunks = H // QUANT_ROWS              # 8 quant chunks per pair
    n_iters = H // (2 * ROWS_PER_TILE)      # 14 conv iterations (8 rows each)
    n_xq = H // XQ_ROWS                     # 7 xq subtiles per pair

    # blocked output layout: (img, blk, ch, iter, row, col).  The store AP is
    # then [(blk ch)=128 partitions, free] -- a 128-wide outer dim splits each
    # store across all 16 SDMA engines.  Host untangles to NCHW.
    y4 = nc.dram_tensor(
        "y4", (IMGS_PER_CORE, 2, COUT, n_iters, ROWS_PER_TILE, W), f16,
        kind="ExternalOutput",
    )

    with tile.TileContext(nc) as tc, ExitStack() as ctx:
        const_pool = ctx.enter_context(tc.tile_pool(name="const", bufs=1))
        xq_pool = ctx.enter_context(tc.tile_pool(name="xq", bufs=2))
        stg_pool = ctx.enter_context(tc.tile_pool(name="stg", bufs=2 * n_parts))
        rnd_pool = ctx.enter_context(tc.tile_pool(name="rnd", bufs=4))
        out_pool = ctx.enter_context(tc.tile_pool(name="out", bufs=4))
        psum_pool = ctx.enter_context(
            tc.tile_pool(name="psum", bufs=8, space=bass.MemorySpace.PSUM)
        )

        seq = []
        for rep in range(repeat):
            for pr in range(IMGS_PER_CORE // 2):
                seq.append((2 * pr, 2 * pr + 1))

        def emit_load(pair, j):
            """Load part j (rows 28j..28j+27) of both images: 201KB DMAs with
            6272B contiguous lines."""
            img_a, img_b = pair
            r0 = j * LOAD_ROWS
            stg = stg_pool.tile([P, LOAD_ROWS, W], f16)
            nc.sync.dma_start(stg[0:64], x4[img_a, :, r0:r0 + LOAD_ROWS, :])
            nc.sync.dma_start(stg[64:128], x4[img_b, :, r0:r0 + LOAD_ROWS, :])
            return stg

        def alloc_xq():
            """7 subtiles of 16 rows (+2 halo) per pair.  Subtile j holds
            padded rows 16j..16j+17 (img row a sits at local a+1-16j).
            Conv iters 2j,2j+1 read subtile j only."""
            tiles = []
            for j in range(n_xq):
                xqj = xq_pool.tile([P, XQ_PAD, HP], bf16)
                nc.vector.memset(xqj[:, :, 0], 128.0)
                nc.vector.memset(xqj[:, :, HP - 1], 128.0)
                if j == 0:
                    nc.vector.memset(xqj[:, 0, :], 128.0)
                if j == n_xq - 1:
                    nc.vector.memset(xqj[:, XQ_PAD - 1, :], 128.0)
                tiles.append(xqj)
            return tiles

        def emit_quant(stgs, tiles, ch):
            r0 = ch * QUANT_ROWS
            r1 = r0 + QUANT_ROWS - 1
            stg = stgs[ch // 2]
            s0 = (ch % 2) * QUANT_ROWS
            rnd = rnd_pool.tile([P, QUANT_ROWS, W], i16)
            # i16 = RNE(64*x + 128): hw f16->i16 cast rounds to nearest even
            nc.gpsimd.tensor_scalar(out=rnd[:], in0=stg[:, s0:s0 + QUANT_ROWS, :],
                                    scalar1=64.0, scalar2=128.0,
                                    op0=Alu.mult, op1=Alu.add)
            # bf16 = clip(i16, 64, 192) == x_int + 128, into each overlapping
            # xq subtile (halo rows are written into two subtiles).  DVE only:
            # gpsimd is ~8x slower on the strided write into the padded tile.
            eng = nc.vector
            for j in range(n_xq):
                lo = max(r0, 16 * j - 1)
                hi = min(r1, 16 * j + 16, H - 1)
                if lo > hi:
                    continue
                eng.tensor_scalar(
                    out=tiles[j][:, lo + 1 - 16 * j:hi + 2 - 16 * j, 1:1 + W],
                    in0=rnd[:, lo - r0:hi - r0 + 1, :],
                    scalar1=64, scalar2=192, op0=Alu.max, op1=Alu.min,
                )

        def emit_conv_iter(pair, tiles, it, o_pair, store_eng):
            img_a, img_b = pair
            xq = tiles[it // 2]
            base = 8 * (it % 2)              # local padded row of output r0
            # ps[0] = image A (blk r in parts 0-63, blk r+1 in parts 64-127),
            # ps[1] = image B likewise.
            ps = []
            for _q in range(2):
                pq = psum_pool.tile([P, NFREE], f32)
                ps.append(pq)
            for tap in range(9):
                dh, dw = divmod(tap, 3)
                st, sp = tap == 0, tap == 8
                for n_img in range(2):           # array row half (image)
                    r = 64 * n_img
                    for blk in range(2):         # array col half (row block)
                        c = 64 * blk
                        hs = base + ROWS_PER_TILE * blk + dh
                        nc.tensor.matmul(
                            ps[n_img][c:c + 64, :],
                            w_t[r:r + 64, tap * 64:(tap + 1) * 64],
                            xq[r:r + 64, hs:hs + ROWS_PER_TILE, dw:dw + W],
                            start=st, stop=sp,
                        )
            # epilogue into half of a 2-iteration output tile; store 16 rows
            # (229KB) once both halves are done.
            half = it % 2
            for n_img, img in enumerate((img_a, img_b)):
                o = o_pair[n_img]
                if n_img == 0:
                    nc.vector.tensor_scalar(
                        out=o[:, half, :], in0=ps[0],
                        scalar1=sb_t[:, 0:1], scalar2=sb_t[:, 1:2],
                        op0=Alu.mult, op1=Alu.add,
                    )
                else:
                    nc.scalar.activation(
                        o[:, half, :], ps[1], Act.Identity,
                        scale=sb_t[:, 0:1], bias=sb_t[:, 1:2],
                    )
                if half == 1:
                    dst = y4[img, :, :, it - 1:it + 1, :, :].rearrange(
                        "b c i r w -> (b c) (i r w)"
                    )
                    store_eng.dma_start(dst, o[:])

        def alloc_o():
            oa = out_pool.tile([P, 2, NFREE], f16)
            ob = out_pool.tile([P, 2, NFREE], f16)
            return oa, ob

        # software pipeline: conv(pair k) interleaves with load+quant(pair k+1).
        # Weights/scales ride the scalar (store) ring, which is empty at
        # startup -- on the sync ring they'd queue FIFO behind 1.6MB of
        # pair-0 input loads and gate the first matmul by ~10us.
        w_t = const_pool.tile([P, 9 * COUT], bf16)
        nc.scalar.dma_start(w_t[:], wsb[:])
        sb_t = const_pool.tile([P, 2], f32)
        nc.scalar.dma_start(sb_t[:], sb[:])
        stgs_k = [emit_load(seq[0], j) for j in range(n_parts)]
        tiles_k = alloc_xq()
        for ch in range(n_chunks):
            emit_quant(stgs_k, tiles_k, ch)
        for k, pair in enumerate(seq):
            last = k + 1 >= len(seq)
            stgs_next = [] if not last else None
            tiles_next = alloc_xq() if not last else None
            o_pair = None
            for it in range(n_iters):
                if not last:
                    if it < n_parts:
                        stgs_next.append(emit_load(seq[k + 1], it))
                    if 4 <= it < 4 + n_chunks:
                        emit_quant(stgs_next, tiles_next, it - 4)
                if it % 2 == 0:
                    o_pair = alloc_o()
                # the last pair's stores ride the sync ring (no loads remain
                # on it, and the scalar ring keeps the ACT epilogues flowing)
                emit_conv_iter(pair, tiles_k, it, o_pair,
                               nc.sync if last else nc.scalar)
            stgs_k, tiles_k = stgs_next, tiles_next

    _split_multi_syncs(nc)
    nc.finalize()
    return nc


def _host_prep(w_q, s_exp, bias, act_exp):
    """Weights in lhsT layout (dup on both partition halves) + scale/bias fold."""
    w_half = np.transpose(w_q, (1, 2, 3, 0)).reshape(CIN, 9 * COUT)  # [ci, tap*64+co]
    wsb = np.concatenate([w_half, w_half], axis=0).astype(ml_dtypes.bfloat16)

    s_exp = np.asarray(s_exp).reshape(-1).astype(np.float64)
    scale = np.exp2(float(act_exp) + s_exp)                       # [64]
    wsum = w_q.astype(np.float64).sum(axis=(1, 2, 3))             # [64]
    bias_c = np.asarray(bias).astype(np.float64) - 128.0 * wsum * scale
    col_scale = np.tile(scale, 2).astype(np.float32)
    col_bias = np.tile(bias_c, 2).astype(np.float32)
    sb = np.stack([col_scale, col_bias], axis=1)                  # [128, 2] f32
    return wsb, sb


def kernel(x, w_q, s_exp, bias, act_exp):
    from concourse.bass_utils import run_bass_kernel_spmd

    # fp16 input transfer: quantization rounds x to multiples of 2^-6 with
    # clip at +-1, so the fp16 cast (11-bit mantissa) only perturbs values
    # that sit within ~2^-11 of a rounding boundary; measured end-to-end
    # rel err ~3e-3, well under tolerance.
    x = np.ascontiguousarray(np.asarray(x).astype(np.float16))
    wsb, sb = _host_prep(np.asarray(w_q), s_exp, bias, int(act_exp))

    if "nc" not in _NC_CACHE:
        _NC_CACHE["nc"] = build_nc()
    nc = _NC_CACHE["nc"]

    in_maps = [
        {"x4": x[4 * c:4 * c + 4], "wsb": wsb, "sb": sb}
        for c in range(N_CORES)
    ]
    res = run_bass_kernel_spmd(nc, in_maps, core_ids=list(range(N_CORES)))
    # device layout (img, blk, ch, iter, row, col) -> NCHW with
    # h = iter*8 + blk*4 + row
    out = np.concatenate([res.results[c]["y4"] for c in range(N_CORES)], axis=0)
    n = out.shape[0]
    out = out.transpose(0, 2, 3, 1, 4, 5).reshape(n, COUT, H, W)
    return np.ascontiguousarray(out.astype(np.float32))


# revision 23
# speedup vs baseline: 1.1878x; 1.1878x over previous
"""BitConv2d (ternary-weight 3x3 conv, power-of-two rescale) on 8 TRN2 NeuronCores.

Strategy:
  - Data-parallel over batch: 32 images -> 4 per core (2 image pairs).
  - fp16 input transfer (quantization makes the fp16 cast nearly lossless:
    measured end-to-end rel err ~3e-3 vs 2e-2 tolerance).  Loads arrive in
    28-row parts (201KB DMAs, 6272B lines) into per-part staging subtiles so
    quantization starts after the first part, not the whole image.
  - Activation quantization x_int = clip(round(clip(x,-1,1)/2^-6), -127, 127):
      i16 = RNE(64*x + 128) on GPSIMD (hw cast rounds to nearest even),
      bf16 = clip(i16, 64, 192) -> v = x_int + 128 (exact ints), alternating
      DVE / GPSIMD per chunk to balance engine load.
    The +128 offset keeps values positive; padded border cells are memset to
    128 so the offset contributes exactly 128*sum(w) per output channel,
    which is folded into the bias on the host.  xq is split into 7 subtiles
    of 16 rows (+2 halo) per pair so conv iterations start as soon as their
    rows are quantized.
  - Conv as 9 accumulating matmuls per output tile (K=Cin=64, M=Cout=64),
    packed 4-per-array with tile_position quadrants:
      rows 0-63   = image A channels, rows 64-127 = image B channels
      cols 0-63   = output row-block r, cols 64-127 = row-block r+1.
    PSUM pairing is per image: ps_A[0:64] = (A, blk r), ps_A[64:128] =
    (A, blk r+1) so the epilogue + store run on full 128-partition tiles.
  - Epilogue y = psum * 2^(act_exp+s_exp[c]) + bias'[c] -> fp16; image A on
    DVE, image B on ACT.  Two iterations share one output tile so stores
    move 16 rows (229KB) each.
  - Loads on the sync-engine HWDGE ring, stores on the scalar-engine ring
    (dedicated directions; mixing them FIFO-blocks the ring).  Stores use a
    blocked dram layout so the AP has a 128-wide outer dim (all 16 SDMA
    engines; a narrow outer dim leaves most engines idle).
  - Teardown sem RANGE_CLEARs distributed across all 5 engines (a single
    engine walks ~25ns/sem -> ~7us serial for ~270 sems).
Output returned as float32 (host upcast + untangle of the fp16 device output).
"""

import numpy as np
import ml_dtypes
from contextlib import ExitStack

_NC_CACHE = {}

N_CORES = 8
H = W = 112
HP = H + 2  # padded width
CIN = COUT = 64
P = 128
IMGS_PER_CORE = 4
LOAD_ROWS = 28                # rows per input load part (4 parts per image)
QUANT_ROWS = 14               # rows per quantization chunk (2 per load part)
ROWS_PER_TILE = 4             # output rows per matmul tile (N = 4*112 = 448)
NFREE = ROWS_PER_TILE * W     # 448
XQ_ROWS = 16                  # img rows per xq subtile (2 conv iters)
XQ_PAD = XQ_ROWS + 2          # + halo


def _patch_bass_and_tile(bass_mod, tile_mod):
    """Build-time patches for this walrus build:
      1. split the final Tile drain's multi-wait into single-wait nops;
      2. distribute semaphore RANGE_CLEARs across all engines (they are
         emitted serially on gpsimd otherwise and cost ~25ns/sem)."""
    from concourse.vector_clock import ScopedClock, VectorClock
    from concourse.bass import compact_to_ranges, SemaphoreHandle

    if getattr(tile_mod.TileContext, "_drain_patched", False):
        return

    def clear_distributed(self, sems):
        if not sems:
            return
        sem_nums = [
            sem.num if isinstance(sem, SemaphoreHandle) else sem for sem in sems
        ]
        sem_ranges = compact_to_ranges(sem_nums)
        # only lightly-loaded engines: a clear emitted on tensor/scalar lands
        # in that engine's instruction stream and stalls matmul/epilogue issue
        # when tile scopes recycle semaphores mid-run.
        engs = [self.sync, self.gpsimd, self.vector]
        per = max(1, (len(sem_nums) + len(engs) - 1) // len(engs))
        ei = cnt = 0
        for sem_range in sem_ranges:
            assert self._state.free_isdisjoint(sem_range)
            x = sem_range.start
            while x < sem_range.stop:
                take = min(sem_range.stop - x, per - cnt)
                sub = range(x, x + take)
                engs[ei].drain(semaphore_range=sub)
                engs[ei].sem_clear(sub)
                x += take
                cnt += take
                if cnt >= per and ei < len(engs) - 1:
                    ei += 1
                    cnt = 0
        self._state.prepend_free_semaphores(sem_nums)
        for poison_set in self._tile_sem_poison_stack:
            poison_set.update(sem_nums)

    bass_mod.Bass.clear_and_free_semaphores = clear_distributed

    def _drain_and_barrier_split(self, tick_clock, wait_clock):
        vclock = tick_clock.global_clock
        n = len(vclock)
        for proc in range(n):
            t = vclock[proc]
            if t <= 0:
                continue
            vec = [0] * n
            vec[proc] = t
            nop = self.nc.sync.nop()
            wait_clock.add_sem_waits(nop.ins, ScopedClock({None: VectorClock(vec)}))
        self.nc.sync.drain()
        assert self.sems is not None
        popped = self.nc._tile_sem_poison_stack.pop()
        assert popped is self._sem_poison
        self.nc.all_engine_barrier()
        self.nc.clear_and_free_semaphores(list(self.sems.allocated().values()))
        self.nc.all_engine_barrier()

    tile_mod.TileContext._drain_and_barrier = _drain_and_barrier_split
    tile_mod.TileContext._drain_patched = True


def _split_multi_syncs(nc):
    """This walrus build accepts at most ONE sync wait (and one update) per
    instruction.  Hoist extra waits onto preceding nops and extra updates onto
    following nops (same engine, so ordering semantics are preserved)."""
    import concourse.mybir as mybir

    fn = nc.m.functions[0]
    ctr = 0
    for bb in fn.blocks:
        new_insts = []
        for inst in bb.instructions:
            si = inst.sync_info
            pre, post = [], []
            if si is not None and si.on_wait and len(si.on_wait) > 1:
                for w in list(si.on_wait[:-1]):
                    ctr += 1
                    pre.append(
                        mybir.InstNoOp(
                            name=f"wsplit_nop_{ctr}",
                            engine=inst.engine,
                            sync_info=mybir.SyncInfo(on_wait=[w], on_update=[]),
                        )
                    )
                si.on_wait = [si.on_wait[-1]]
            if (
                si is not None
                and si.on_update
                and len(si.on_update) > 1
                and not isinstance(inst, (mybir.InstDMACopy, mybir.InstDMA))
            ):
                for u in list(si.on_update[1:]):
                    ctr += 1
                    post.append(
                        mybir.InstNoOp(
                            name=f"usplit_nop_{ctr}",
                            engine=inst.engine,
                            sync_info=mybir.SyncInfo(on_wait=[], on_update=[u]),
                        )
                    )
                si.on_update = [si.on_update[0]]
            new_insts.extend(pre)
            new_insts.append(inst)
            new_insts.extend(post)
        if len(new_insts) != len(bb.instructions):
            bb.instructions[:] = new_insts
    for bb in fn.blocks:
        for inst in bb.instructions:
            if inst.name.startswith(("wsplit_nop_", "usplit_nop_")):
                if inst.name not in nc.inst_map:
                    nc.register_instruction(inst)
    return ctr


def build_nc(repeat: int = 1):
    import concourse.bass as bass
    import concourse.mybir as mybir
    import concourse.tile as tile

    _patch_bass_and_tile(bass, tile)

    f32 = mybir.dt.float32
    f16 = mybir.dt.float16
    bf16 = mybir.dt.bfloat16
    i16 = mybir.dt.int16
    Alu = mybir.AluOpType
    Act = mybir.ActivationFunctionType

    nc = bass.Bass(trn_type="TRN2")
    x4 = nc.dram_tensor("x4", (IMGS_PER_CORE, CIN, H, W), f16, kind="ExternalInput")
    wsb = nc.dram_tensor("wsb", (P, 9 * COUT), bf16, kind="ExternalInput")
    sb = nc.dram_tensor("sb", (P, 2), f32, kind="ExternalInput")

    n_parts = H // LOAD_ROWS                # 4 load parts per image
    n_chunks = H // QUANT_ROWS              # 8 quant chunks per pair
    n_iters = H // (2 * ROWS_PER_TILE)      # 14 conv iterations (8 rows each)
    n_xq = H // XQ_ROWS                     # 7 xq subtiles per pair

    # blocked output layout: (img, blk, ch, iter, row, col).  The store AP is
    # then [(blk ch)=128 partitions, free] -- a 128-wide outer dim splits each
    # store across all 16 SDMA engines.  Host untangles to NCHW.
    y4 = nc.dram_tensor(
        "y4", (IMGS_PER_CORE, 2, COUT, n_iters, ROWS_PER_TILE, W), f16,
        kind="ExternalOutput",
    )

    with tile.TileContext(nc) as tc, ExitStack() as ctx:
        const_pool = ctx.enter_context(tc.tile_pool(name="const", bufs=1))
        xq_pool = ctx.enter_context(tc.tile_pool(name="xq", bufs=2))
        stg_pool = ctx.enter_context(tc.tile_pool(name="stg", bufs=2 * n_parts))
        rnd_pool = ctx.enter_context(tc.tile_pool(name="rnd", bufs=4))
        out_pool = ctx.enter_context(tc.tile_pool(name="out", bufs=4))
        psum_pool = ctx.enter_context(
            tc.tile_pool(name="psum", bufs=8, space=bass.MemorySpace.PSUM)
        )

        seq = []
        for rep in range(repeat):
            for pr in range(IMGS_PER_CORE // 2):
                seq.append((2 * pr, 2 * pr + 1))

        def emit_load(pair, j):
            """Load part j (rows 28j..28j+27) of both images: 201KB DMAs with
            6272B contiguous lines."""
            img_a, img_b = pair
            r0 = j * LOAD_ROWS
            stg = stg_pool.tile([P, LOAD_ROWS, W], f16)
            nc.sync.dma_start(stg[0:64], x4[img_a, :, r0:r0 + LOAD_ROWS, :])
            nc.sync.dma_start(stg[64:128], x4[img_b, :, r0:r0 + LOAD_ROWS, :])
            return stg

        def alloc_xq():
            """7 subtiles of 16 rows (+2 halo) per pair.  Subtile j holds
            padded rows 16j..16j+17 (img row a sits at local a+1-16j).
            Conv iters 2j,2j+1 read subtile j only."""
            tiles = []
            for j in range(n_xq):
                xqj = xq_pool.tile([P, XQ_PAD, HP], bf16)
                nc.vector.memset(xqj[:, :, 0], 128.0)
                nc.vector.memset(xqj[:, :, HP - 1], 128.0)
                if j == 0:
                    nc.vector.memset(xqj[:, 0, :], 128.0)
                if j == n_xq - 1:
                    nc.vector.memset(xqj[:, XQ_PAD - 1, :], 128.0)
                tiles.append(xqj)
            return tiles

        def emit_quant(stgs, tiles, ch):
            r0 = ch * QUANT_ROWS
            r1 = r0 + QUANT_ROWS - 1
            stg = stgs[ch // 2]
            s0 = (ch % 2) * QUANT_ROWS
            rnd = rnd_pool.tile([P, QUANT_ROWS, W], i16)
            # i16 = RNE(64*x + 128): hw f16->i16 cast rounds to nearest even
            nc.gpsimd.tensor_scalar(out=rnd[:], in0=stg[:, s0:s0 + QUANT_ROWS, :],
                                    scalar1=64.0, scalar2=128.0,
                                    op0=Alu.mult, op1=Alu.add)
            # bf16 = clip(i16, 64, 192) == x_int + 128, into each overlapping
            # xq subtile (halo rows are written into two subtiles).  DVE only:
            # gpsimd is ~8x slower on the strided write into the padded tile.
            eng = nc.vector
            for j in range(n_xq):
                lo = max(r0, 16 * j - 1)
                hi = min(r1, 16 * j + 16, H - 1)
                if lo > hi:
                    continue
                eng.tensor_scalar(
                    out=tiles[j][:, lo + 1 - 16 * j:hi + 2 - 16 * j, 1:1 + W],
                    in0=rnd[:, lo - r0:hi - r0 + 1, :],
                    scalar1=64, scalar2=192, op0=Alu.max, op1=Alu.min,
                )

        def emit_conv_iter(pair, tiles, it, o_pair, store_eng):
            img_a, img_b = pair
            xq = tiles[it // 2]
            base = 8 * (it % 2)              # local padded row of output r0
            # ps[0] = image A (blk r in parts 0-63, blk r+1 in parts 64-127),
            # ps[1] = image B likewise.
            ps = []
            for _q in range(2):
                pq = psum_pool.tile([P, NFREE], f32)
                ps.append(pq)
            for tap in range(9):
                dh, dw = divmod(tap, 3)
                st, sp = tap == 0, tap == 8
                for n_img in range(2):           # array row half (image)
                    r = 64 * n_img
                    for blk in range(2):         # array col half (row block)
                        c = 64 * blk
                        hs = base + ROWS_PER_TILE * blk + dh
                        nc.tensor.matmul(
                            ps[n_img][c:c + 64, :],
                            w_t[r:r + 64, tap * 64:(tap + 1) * 64],
                            xq[r:r + 64, hs:hs + ROWS_PER_TILE, dw:dw + W],
                            start=st, stop=sp,
                        )
            # epilogue into half of a 2-iteration output tile; store 16 rows
            # (229KB) once both halves are done.
            half = it % 2
            for n_img, img in enumerate((img_a, img_b)):
                o = o_pair[n_img]
                if n_img == 0:
                    nc.vector.tensor_scalar(
                        out=o[:, half, :], in0=ps[0],
                        scalar1=sb_t[:, 0:1], scalar2=sb_t[:, 1:2],
                        op0=Alu.mult, op1=Alu.add,
                    )
                else:
                    nc.scalar.activation(
                        o[:, half, :], ps[1], Act.Identity,
                        scale=sb_t[:, 0:1], bias=sb_t[:, 1:2],
                    )
                if half == 1:
                    dst = y4[img, :, :, it - 1:it + 1, :, :].rearrange(
                        "b c i r w -> (b c) (i r w)"
                    )
                    store_eng.dma_start(dst, o[:])

        def alloc_o():
            oa = out_pool.tile([P, 2, NFREE], f16)
            ob = out_pool.tile([P, 2, NFREE], f16)
            return oa, ob

        # software pipeline: conv(pair k) interleaves with load+quant(pair k+1).
        # Weights/scales ride the scalar (store) ring, which is empty at
        # startup -- on the sync ring they'd queue FIFO behind 1.6MB of
        # pair-0 input loads and gate the first matmul by ~10us.
        w_t = const_pool.tile([P, 9 * COUT], bf16)
        nc.scalar.dma_start(w_t[:], wsb[:])
        sb_t = const_pool.tile([P, 2], f32)
        nc.scalar.dma_start(sb_t[:], sb[:])
        stgs_k = [emit_load(seq[0], j) for j in range(n_parts)]
        tiles_k = alloc_xq()
        for ch in range(n_chunks):
            emit_quant(stgs_k, tiles_k, ch)
        for k, pair in enumerate(seq):
            last = k + 1 >= len(seq)
            stgs_next = [] if not last else None
            tiles_next = alloc_xq() if not last else None
            o_pair = None
            for it in range(n_iters):
                if not last:
                    if it < n_parts:
                        stgs_next.append(emit_load(seq[k + 1], it))
                    if 4 <= it < 4 + n_chunks:
                        emit_quant(stgs_next, tiles_next, it - 4)
                if it % 2 == 0:
                    o_pair = alloc_o()
                emit_conv_iter(pair, tiles_k, it, o_pair, nc.scalar)
            stgs_k, tiles_k = stgs_next, tiles_next

    _split_multi_syncs(nc)
    nc.finalize()
    return nc


def _host_prep(w_q, s_exp, bias, act_exp):
    """Weights in lhsT layout (dup on both partition halves) + scale/bias fold."""
    w_half = np.transpose(w_q, (1, 2, 3, 0)).reshape(CIN, 9 * COUT)  # [ci, tap*64+co]
    wsb = np.concatenate([w_half, w_half], axis=0).astype(ml_dtypes.bfloat16)

    s_exp = np.asarray(s_exp).reshape(-1).astype(np.float64)
    scale = np.exp2(float(act_exp) + s_exp)                       # [64]
    wsum = w_q.astype(np.float64).sum(axis=(1, 2, 3))             # [64]
    bias_c = np.asarray(bias).astype(np.float64) - 128.0 * wsum * scale
    col_scale = np.tile(scale, 2).astype(np.float32)
    col_bias = np.tile(bias_c, 2).astype(np.float32)
    sb = np.stack([col_scale, col_bias], axis=1)                  # [128, 2] f32
    return wsb, sb


def kernel(x, w_q, s_exp, bias, act_exp):
    from concourse.bass_utils import run_bass_kernel_spmd

    # fp16 input transfer: quantization rounds x to multiples of 2^-6 with
    # clip at +-1, so the fp16 cast (11-bit mantissa) only perturbs values
    # that sit within ~2^-11 of a rounding boundary; measured end-to-end
    # rel err ~3e-3, well under tolerance.
    x = np.ascontiguousarray(np.asarray(x).astype(np.float16))
    wsb, sb = _host_prep(np.asarray(w_q), s_exp, bias, int(act_exp))

    if "nc" not in _NC_CACHE:
        _NC_CACHE["nc"] = build_nc()
    nc = _NC_CACHE["nc"]

    in_maps = [
        {"x4": x[4 * c:4 * c + 4], "wsb": wsb, "sb": sb}
        for c in range(N_CORES)
    ]
    res = run_bass_kernel_spmd(nc, in_maps, core_ids=list(range(N_CORES)))
    # device layout (img, blk, ch, iter, row, col) -> NCHW with
    # h = iter*8 + blk*4 + row
    out = np.concatenate([res.results[c]["y4"] for c in range(N_CORES)], axis=0)
    n = out.shape[0]
    out = out.transpose(0, 2, 3, 1, 4, 5).reshape(n, COUT, H, W)
    return np.ascontiguousarray(out.astype(np.float32))
